# revision 8
# baseline (speedup 1.0000x reference)
"""AFNO2D (channel-first) Trainium2 kernel, v3.

out = x + irfft2( softshrink(mlp2(leaky(mlp1(rfft2(x))))) * rfft2(x) )
with block-diagonal complex MLPs over 8 channel blocks of 96.

Sharding: block-parallel - core k owns spectral block k (96 channels), zero
collectives.

v3 design vs v2 (593us):
 - S2 and mix2T use fp8e4 DoubleRow matmuls: each DR instruction computes the
   complex-accumulate PAIR (two K=128/97 products summed) at 0.5 cyc/out-col,
   a 4x PE cut on those stages.  u1 and o1 are staged fp8; scale factors are
   folded into host constants to keep every fp8 tensor in range:
     fw has no 1/n -> u1 = DFT_W(x), sigma~8
     s2pair = twiddles/4  -> utr = 32*xf (bf16, max ~160 < 240 fp8 if it were)
     w1 consts /32 (bias row unscaled) -> o1 = leaky(W1 xf + b1) exact
     w2pair = 32*W2 (bias 32*b2), lambda' = 0.32 -> sh = 32*sh_ref
     ybig = sh (.) utT = 1024*y_ref (bf16), fic/fis /1024 -> exact out
 - shrink restructured: ACT drains mix2-psum to bf16 o2t once; clamp and
   subtract then run on DVE in 4x perf mode (all-SBUF bf16) instead of two
   1x psum-source passes.
 - gate arithmetic batched at nq=4 (half the instruction count + semaphores);
   ta/ybig-re on DVE (4x), tb/ybig-im on GpSimd as before.
 - utr drain pinned to ACT, vdrain alternates ACT/DVE, s1 drain ACT (fp8 out).
 - ones rows of utr/o1 memset once at start (tiles persist across batches).

PSUM rule: matmul start=True clears has_written for the WHOLE bank, so each
bank gets exactly one start=True (its first matmul); all later matmuls use
start=False (fresh ranges overwrite, accumulation ranges add).

Hardcoded shapes: x [4,768,128,128] f32, w1/w2 [2,8,96,96], b1/b2 [2,8,96].
"""

import os
import numpy as np
import ml_dtypes

B, C, H, W = 4, 768, 128, 128
NBLK, BS = 8, 96          # spectral blocks, channels per core
WF = 65                   # rfft size along W
LAM = 0.01                # softshrink threshold
NS = 0.1                  # leaky relu negative slope

SH_SCALE = 32.0           # sh = 32*sh_ref ; lambda' = 32*lam
U_SCALE = 32.0            # utr/utT = 32*xf
LAMS = LAM * SH_SCALE

BF16 = ml_dtypes.bfloat16
FP8 = ml_dtypes.float8_e4m3

LAST_RESULT = {}          # diagnostics (exec_time_ns) for the test harness


def _twiddles():
    n = 128
    wv = np.arange(n)[:, None].astype(np.float64)
    jv = np.arange(n)[None, :].astype(np.float64)
    ang = 2.0 * np.pi * wv * jv / n  # [128,128]

    # S1 moving operand [w, 130]: cols 0..64 cos ; cols 65..129 -sin
    # (imag cols 65 and 129 i.e. wf=0,64 are exactly zero).  No 1/n: u1 is
    # the unnormalized DFT_W so the fp8 staging sits at sigma~8.
    fw = np.zeros((n, 130), np.float64)
    fw[:, :WF] = np.cos(ang[:, :WF])
    fw[:, WF + 1:WF + 64] = -np.sin(ang[:, 1:64])

    c = np.cos(ang)
    s = np.sin(ang)
    # S2 DoubleRow moving [128, 2, 256]: plane 0 pairs with u1-re, plane 1
    # with u1-im.  /4 so utr = DFT2(x)/4 = 32*xf.
    s2pair = np.stack([np.hstack([c, -s]), np.hstack([s, c])], axis=1) / 4.0
    mcs = np.hstack([c, s])       # iDFT moving for Yr: [cos | sin]
    msc = np.hstack([-s, c])      # iDFT moving for Yi: [-sin | cos]

    # final irfft_W stationaries; /1024 compensates ybig = 1024*y_ref
    alpha = np.full((WF, 1), 2.0)
    alpha[0, 0] = 1.0
    alpha[64, 0] = 1.0
    fic = alpha * np.cos(2.0 * np.pi * np.arange(WF)[:, None]
                         * np.arange(n)[None, :] / n) / n / 1024.0
    fis64 = np.zeros((64, n), np.float64)                   # row 0 (wf=0) = 0
    fis64[1:64] = -2.0 * np.sin(2.0 * np.pi * np.arange(1, 64)[:, None]
                                * np.arange(n)[None, :] / n) / n / 1024.0

    return (fw.astype(BF16), s2pair.astype(FP8),
            mcs.astype(BF16), msc.astype(BF16),
            fic.astype(BF16), fis64.astype(BF16))


def _build():
    import concourse.mybir as mybir
    import concourse.tile as tile
    import concourse.bass as _bass
    from concourse import bacc

    dt = mybir.dt
    AF = mybir.ActivationFunctionType
    ALU = mybir.AluOpType
    DR = mybir.MatmulPerfMode.DoubleRow

    nc = bacc.Bacc("TRN2", target_bir_lowering=False, debug=False)

    xt = nc.declare_dram_parameter("xt", [B, W, BS, H], dt.bfloat16, isOutput=False)
    # out is [b, c, w, h]; host transposes the last two axes back
    out = nc.declare_dram_parameter("out", [B, BS, W, H], dt.bfloat16, isOutput=True)

    fw_d = nc.declare_dram_parameter("fw", [128, 130], dt.bfloat16, isOutput=False)
    s2p_d = nc.declare_dram_parameter("s2pair", [128, 2, 256], dt.float8e4, isOutput=False)
    mcs_d = nc.declare_dram_parameter("mcs", [128, 256], dt.bfloat16, isOutput=False)
    msc_d = nc.declare_dram_parameter("msc", [128, 256], dt.bfloat16, isOutput=False)
    fic_d = nc.declare_dram_parameter("fic", [WF, 128], dt.bfloat16, isOutput=False)
    fis_d = nc.declare_dram_parameter("fis64", [64, 128], dt.bfloat16, isOutput=False)

    wnames = ("w1ra", "w1ia", "w1in", "w1r_")
    wshapes = {"w1ra": [BS + 1, 128], "w1ia": [BS + 1, 128],
               "w1in": [BS, 128], "w1r_": [BS, 128]}
    wds = {nm: nc.declare_dram_parameter(nm, wshapes[nm], dt.bfloat16,
                                         isOutput=False)
           for nm in wnames}
    w2p_d = nc.declare_dram_parameter("w2pair", [BS + 1, 2, 192], dt.float8e4,
                                      isOutput=False)

    dbg = bool(int(os.environ.get("AFNO_DEBUG", "0")))
    if dbg:
        d_u1 = nc.declare_dram_parameter("d_u1", [128, WF, 2, BS], dt.float8e4, isOutput=True)
        d_utr = nc.declare_dram_parameter("d_utr", [BS + 1, 2, WF, 128], dt.bfloat16, isOutput=True)
        d_utT = nc.declare_dram_parameter("d_utT", [128, 2, WF, BS], dt.bfloat16, isOutput=True)
        d_o1 = nc.declare_dram_parameter("d_o1", [BS + 1, 2, 512], dt.float8e4, isOutput=True)
        d_o2 = nc.declare_dram_parameter("d_o2", [128, 4, 192], dt.bfloat16, isOutput=True)
        d_ybig = nc.declare_dram_parameter("d_ybig", [128, 2, WF, BS], dt.bfloat16, isOutput=True)
        d_vst = nc.declare_dram_parameter("d_vst", [WF, 4, 256], dt.bfloat16, isOutput=True)

    with tile.TileContext(nc) as tc:
        with (
            tc.tile_pool(name="consts", bufs=1) as consts,
            tc.tile_pool(name="xts", bufs=4) as xts_p,
            tc.tile_pool(name="u1", bufs=1) as u1_p,
            tc.tile_pool(name="utr", bufs=1) as utr_p,
            tc.tile_pool(name="utT", bufs=1) as utT_p,
            tc.tile_pool(name="ybig", bufs=1) as ybig_p,
            tc.tile_pool(name="o1", bufs=3) as o1_p,
            tc.tile_pool(name="o2t", bufs=2) as o2t_p,
            tc.tile_pool(name="sg", bufs=3) as sg_p,
            tc.tile_pool(name="vst", bufs=2) as vst_p,
            tc.tile_pool(name="ot", bufs=3) as ot_p,
            tc.tile_pool(name="pS", bufs=2, space="PSUM") as pS,
            tc.tile_pool(name="pM", bufs=2, space="PSUM") as pM,
            tc.tile_pool(name="pN", bufs=2, space="PSUM") as pN,
        ):
            # ---------------- constants ----------------
            def cload(dparam, shape, tag, dtype=dt.bfloat16):
                t = consts.tile(shape, dtype, tag=tag, name=tag)
                nc.sync.dma_start(out=t[:], in_=dparam[:])
                return t

            fw = cload(fw_d, [128, 130], "fw")
            s2pair = cload(s2p_d, [128, 2, 256], "s2pair", dt.float8e4)
            mcs = cload(mcs_d, [128, 256], "mcs")
            msc = cload(msc_d, [128, 256], "msc")
            fic = cload(fic_d, [WF, 128], "fic")
            fis64 = cload(fis_d, [64, 128], "fis64")
            wt = {nm: cload(wds[nm], wshapes[nm], nm) for nm in wnames}
            w2pair = cload(w2p_d, [BS + 1, 2, 192], "w2pair", dt.float8e4)

            # persistent tiles: ones/zero rows set once (pools rotate over
            # fixed buffers; drains never touch these rows)
            utr_t = utr_p.tile([BS + 1, 2, WF, 128], dt.bfloat16, tag="utr")
            nc.gpsimd.memset(utr_t[BS:BS + 1, 0, :, :], 1.0)
            nc.gpsimd.memset(utr_t[BS:BS + 1, 1, :, :], 0.0)
            o1_pre = [o1_p.tile([BS + 1, 2, 512], dt.float8e4, tag="o1",
                                name="o1") for _ in range(3)]
            for t in o1_pre:
                nc.gpsimd.memset(t[BS:BS + 1, 0, :], 1.0)
                nc.gpsimd.memset(t[BS:BS + 1, 1, :], 0.0)

            def load_x(b):
                xh = []
                for hh in range(2):
                    t = xts_p.tile([128, 48, 128], dt.bfloat16, tag="xts", name="xts")
                    nc.sync.dma_start(out=t[:], in_=xt[b, :, hh * 48:(hh + 1) * 48, :])
                    xh.append(t)
                return xh

            def s1(b, xh):
                # u1 layout [h, j, (re,im), c]: the S2 DoubleRow stationary
                # must have a contiguous innermost dim (ISA
                # s3_lw_dual_fp8_restrictions), so channels go innermost and
                # the re/im pair sits just above them.
                u1 = u1_p.tile([128, WF, 2, BS], dt.float8e4, tag="u1")
                for g in range(BS // 3):
                    ps = pS.tile([128, 3, 130], dt.float32, tag="pS", name="ps1")
                    for k in range(3):
                        c = 3 * g + k
                        lhs = xh[c // 48][:, c % 48, :]
                        nc.tensor.matmul(ps[:, k, :], lhs, fw,
                                         start=(k == 0), stop=(k == 2),
                                         skip_group_check=True)
                    nc.scalar.activation(
                        u1[:, :, :, 3 * g:3 * g + 3].rearrange(
                            "p j r c -> p c r j"),
                        ps[:, :, :].rearrange("p c (r j) -> p c r j", r=2),
                        AF.Copy, bias=0.0, scale=1.0)
                if dbg and b == 0:
                    nc.sync.dma_start(out=d_u1[:, :, :, :], in_=u1[:, :, :, :])
                return u1

            def s2(b, u1):
                # DFT along H: one fp8 DoubleRow matmul per frequency j
                # (stationary = (u1-re[,j], u1-im[,j]) pair, moving = s2pair).
                utr = utr_p.tile([BS + 1, 2, WF, 128], dt.bfloat16, tag="utr")
                utT = utT_p.tile([128, 2, WF, BS], dt.bfloat16, tag="utT")
                u1a = u1[:, :, :]
                xbar_after = {11: (0, 24), 23: (24, 48), 32: (48, WF)}
                for jp in range(33):
                    js = [j for j in (2 * jp, 2 * jp + 1) if j < WF]
                    ps = pS.tile([128, 2, 256], dt.float32, tag="pS", name="ps2")
                    for q, j in enumerate(js):
                        lhsT = _bass.AP(tensor=u1a.tensor,
                                        offset=u1a.offset + 2 * BS * j,
                                        ap=[u1a.ap[0], [BS, 2], [1, BS]])
                        nc.tensor.matmul(ps[0:BS, q, :], lhsT, s2pair[:, :, :],
                                         start=(q == 0), stop=(q == len(js) - 1),
                                         perf_mode=DR, skip_group_check=True)
                    src = ps[0:BS, 0:len(js), :].rearrange(
                        "p q (r h) -> p q r h", r=2)
                    dst = utr[0:BS, :, js[0]:js[0] + len(js), :].rearrange(
                        "p r j h -> p j r h")
                    if jp % 2 == 0:
                        nc.scalar.activation(dst, src, AF.Copy, bias=0.0,
                                             scale=1.0)
                    else:
                        nc.vector.tensor_copy(dst, src)
                    if jp in xbar_after:
                        lo, hi = xbar_after[jp]
                        for ri in range(2):
                            nc.sync.dma_start_transpose(
                                out=utT[:, ri, lo:hi, :],
                                in_=utr[0:BS, ri, lo:hi, :])
                if dbg and b == 0:
                    nc.sync.dma_start(out=d_utr[:, :, :, :], in_=utr[:, :, :, :])
                    nc.sync.dma_start(out=d_utT[:, :, :, :], in_=utT[:, :, :, :])
                return utr, utT

            def mix(b, utr, utT):
                ybig = ybig_p.tile([128, 2, WF, BS], dt.bfloat16, tag="ybig")
                nchunk = 17                   # 16 x 512 + 1 x 128

                def mix1(ci):
                    j0 = 4 * ci
                    njj = min(4, WF - j0)
                    sz = njj * 128
                    ur = utr[0:BS + 1, 0, j0:j0 + njj, :]
                    ui = utr[0:BS, 1, j0:j0 + njj, :]
                    p1 = pM.tile([128, 1024], dt.float32, tag="pM", name="p1")
                    nc.tensor.matmul(p1[:, 0:sz], wt["w1ra"], ur,
                                     start=True, stop=False, skip_group_check=True)
                    nc.tensor.matmul(p1[:, 512:512 + sz], wt["w1ia"], ur,
                                     start=True, stop=False, skip_group_check=True)
                    nc.tensor.matmul(p1[:, 0:sz], wt["w1in"], ui,
                                     start=False, stop=False, skip_group_check=True)
                    nc.tensor.matmul(p1[:, 512:512 + sz], wt["w1r_"], ui,
                                     start=False, stop=True, skip_group_check=True)
                    o1 = o1_p.tile([BS + 1, 2, 512], dt.float8e4, tag="o1",
                                   name="o1")
                    psrc = p1[0:BS, :].rearrange("p (r f) -> p r f", r=2)[:, :, 0:sz]
                    nc.scalar.activation(o1[0:BS, :, 0:sz], psrc, AF.Prelu,
                                         bias=0.0, scale=1.0, alpha=NS)
                    return (ci, j0, njj, o1)

                def mix2T(st):
                    ci, j0, njj, o1 = st
                    if dbg and b == 0 and ci == 0:
                        nc.sync.dma_start(out=d_o1[:, :, :], in_=o1[:, :, :])
                    o2t = o2t_p.tile([128, 4, 192], dt.bfloat16, tag="o2t",
                                     name="o2t")
                    o1a = o1[:, :, :]
                    for t0 in range(0, njj, 2):
                        js = [jj for jj in (t0, t0 + 1) if jj < njj]
                        p2 = pN.tile([128, 2, 192], dt.float32, tag="pN",
                                     name="p2")
                        for q, jj in enumerate(js):
                            lhsT = _bass.AP(
                                tensor=o1a.tensor, offset=o1a.offset + 128 * jj,
                                ap=[o1a.ap[0], [512, 2], [1, 128]])
                            nc.tensor.matmul(p2[:, q, :], lhsT, w2pair[:, :, :],
                                             start=(q == 0), stop=(q == len(js) - 1),
                                             perf_mode=DR, skip_group_check=True)
                        nq = len(js)
                        nc.scalar.activation(o2t[:, t0:t0 + nq, :], p2[:, 0:nq, :],
                                             AF.Copy, bias=0.0, scale=1.0)
                    if dbg and b == 0 and ci == 0:
                        nc.sync.dma_start(out=d_o2[:, :, :], in_=o2t[:, :, :])
                    # softshrink on bf16 SBUF (DVE 4x): sh = o2t - clamp(o2t)
                    nq = njj
                    cl = sg_p.tile([128, 4, 192], dt.bfloat16, tag="cl",
                                   name="cl")
                    nc.vector.tensor_scalar(cl[:, 0:nq, :], o2t[:, 0:nq, :],
                                            -LAMS, LAMS, ALU.max, ALU.min)
                    sh = sg_p.tile([128, 4, 192], dt.bfloat16, tag="sh",
                                   name="sh")
                    nc.vector.tensor_sub(sh[:, 0:nq, :], o2t[:, 0:nq, :],
                                         cl[:, 0:nq, :])
                    # gate: y = s * u (complex), in [hf, c] layout, nq j's at once
                    _s = sh[:, :, :]
                    sr2 = _bass.AP(
                        tensor=_s.tensor, offset=_s.offset,
                        ap=[_s.ap[0], [192, nq], [0, 2], [1, BS]])
                    si2 = _bass.AP(
                        tensor=_s.tensor, offset=_s.offset + 96,
                        ap=[_s.ap[0], [192, nq], [0, 2], [1, BS]])
                    u2 = utT[:, :, j0:j0 + nq, :].rearrange(
                        "p r j c -> p j r c")
                    ta = sg_p.tile([128, 4, 2, BS], dt.bfloat16, tag="ta",
                                   name="ta")
                    tb = sg_p.tile([128, 4, 2, BS], dt.bfloat16, tag="tb",
                                   name="tb")
                    nc.vector.tensor_mul(ta[:, 0:nq, :, :], sr2, u2)
                    nc.gpsimd.tensor_mul(tb[:, 0:nq, :, :], si2, u2)
                    nc.vector.tensor_sub(
                        ybig[:, 0, j0:j0 + nq, :],
                        ta[:, 0:nq, 0, :], tb[:, 0:nq, 1, :])
                    nc.gpsimd.tensor_add(
                        ybig[:, 1, j0:j0 + nq, :],
                        ta[:, 0:nq, 1, :], tb[:, 0:nq, 0, :])

                prev = None
                for ci in range(nchunk):
                    cur = mix1(ci)
                    if prev is not None:
                        mix2T(prev)
                    prev = cur
                mix2T(prev)
                if dbg and b == 0:
                    nc.sync.dma_start(out=d_ybig[:, :, :, :], in_=ybig[:, :, :, :])
                return ybig

            def idft_final(b, ybig, xh):
                def idftA(p):
                    pV = pN.tile([128, 2, 256], dt.float32, tag="pN", name="pV")
                    for cc in range(2):
                        c = 2 * p + cc
                        yr = ybig[:, 0, :, c]
                        yi = ybig[:, 1, :, c]
                        nc.tensor.matmul(pV[0:WF, cc, :], yr, mcs,
                                         start=(cc == 0), stop=False,
                                         skip_group_check=True)
                        nc.tensor.matmul(pV[0:WF, cc, :], yi, msc,
                                         start=False, stop=(cc == 1),
                                         skip_group_check=True)
                    return pV

                def vdrain(p, pV, vst):
                    q = p % 2
                    dst = vst[0:WF, 2 * q:2 * q + 2, :]
                    src = pV[0:WF, :, :]
                    if p % 2 == 0:
                        nc.scalar.activation(dst, src, AF.Copy, bias=0.0,
                                             scale=1.0)
                    else:
                        nc.vector.tensor_copy(dst, src)

                def final(qd, vst):
                    pO = pM.tile([128, 4, 128], dt.float32, tag="pM", name="pO")
                    nc.tensor.matmul(pO[:, :, :], fic, vst[0:WF, :, 0:128],
                                     start=True, stop=False,
                                     skip_group_check=True)
                    nc.tensor.matmul(pO[:, :, :], fis64, vst[0:64, :, 128:256],
                                     start=False, stop=True,
                                     skip_group_check=True)
                    c0 = 4 * qd
                    hh = c0 // 48
                    xh4 = xh[hh][:, c0 % 48:c0 % 48 + 4, :]
                    ot = ot_p.tile([128, 4, 128], dt.bfloat16, tag="ot",
                                   name="ot")
                    nc.vector.tensor_add(
                        ot[:, :, :].rearrange("p c f -> p (c f)"),
                        pO[:, :, :].rearrange("p c f -> p (c f)"),
                        xh4.rearrange("p c f -> p (c f)"))
                    nc.sync.dma_start(
                        out=out[b, c0:c0 + 4, :, :].rearrange("c w h -> w c h"),
                        in_=ot[:, :, :])

                vsts = {}
                for p in range(BS // 2):
                    qd = p // 2
                    if p % 2 == 0:
                        vsts[qd] = vst_p.tile([WF, 4, 256], dt.bfloat16,
                                              tag="vst", name="vst")
                    pV = idftA(p)
                    vdrain(p, pV, vsts[qd])
                    if dbg and b == 0 and p == 1:
                        nc.sync.dma_start(out=d_vst[:, :, :], in_=vsts[0][:, :, :])
                    if p % 2 == 1 and p >= 3:
                        final(qd - 1, vsts[qd - 1])
                final(BS // 4 - 1, vsts[BS // 4 - 1])

            # ---- batch-level software pipeline: S1(b+1) fills the PE while
            # ---- the mix(b) elementwise tail drains; S2(b+1) follows idft(b).
            xh_all = {0: load_x(0)}
            u1 = s1(0, xh_all[0])
            utr, utT = s2(0, u1)
            for b in range(B):
                if b + 1 < B:
                    xh_all[b + 1] = load_x(b + 1)
                ybig = mix(b, utr, utT)
                if b + 1 < B:
                    u1 = s1(b + 1, xh_all[b + 1])
                idft_final(b, ybig, xh_all[b])
                del xh_all[b]
                if b + 1 < B:
                    utr, utT = s2(b + 1, u1)

    nc.finalize()
    return nc


_BUILT = None


def _get_built():
    global _BUILT
    if _BUILT is None:
        _BUILT = _build()
    return _BUILT


def _make_in_maps(x, w1, b1, w2, b2):
    fw, s2pair, mcs, msc, fic, fis64 = _twiddles()
    in_maps = []
    for k in range(NBLK):
        xs = x[:, k * BS:(k + 1) * BS]
        w1r, w1i = w1[0, k], w1[1, k]
        w2r, w2i = w2[0, k], w2[1, k]

        def pad128(a):
            o = np.zeros((a.shape[0], 128), np.float32)
            o[:, 0:BS] = a
            return o.astype(BF16)

        # mix1 consumes utr = 32*xf: weights /32, bias row unscaled
        inv = 1.0 / U_SCALE
        # mix2 DoubleRow pair: plane0 x o1-re, plane1 x o1-im; x32 so
        # sh = 32*sh_ref and the fp8 weights sit at ~0.6
        w2p0 = np.vstack([np.hstack([w2r, w2i]),
                          np.hstack([b2[0, k][None, :], b2[1, k][None, :]])])
        w2p1 = np.vstack([np.hstack([-w2i, w2r]), np.zeros((1, 192))])
        w2pair = (SH_SCALE * np.stack([w2p0, w2p1], axis=1)).astype(FP8)
        m = {
            "xt": np.ascontiguousarray(xs.transpose(0, 3, 1, 2)).astype(BF16),
            "fw": fw, "s2pair": s2pair, "mcs": mcs, "msc": msc,
            "fic": fic, "fis64": fis64,
            "w1ra": pad128(np.vstack([inv * w1r, b1[0, k][None, :]])),
            "w1ia": pad128(np.vstack([inv * w1i, b1[1, k][None, :]])),
            "w1in": pad128(-inv * w1i), "w1r_": pad128(inv * w1r),
            "w2pair": w2pair,
        }
        in_maps.append(m)
    return in_maps


def kernel(x, w1, b1, w2, b2):
    from concourse.bass_utils import run_bass_kernel_spmd

    nc = _get_built()
    in_maps = _make_in_maps(x, w1, b1, w2, b2)

    trace = bool(int(os.environ.get("AFNO_TRACE", "0")))
    kw = {}
    if trace:
        import tempfile
        kw["tmpdir"] = tempfile.mkdtemp(prefix="afno_trace_")
        LAST_RESULT["trace_dir"] = kw["tmpdir"]
    res = run_bass_kernel_spmd(nc, in_maps, core_ids=list(range(NBLK)),
                               trace=trace, **kw)
    LAST_RESULT["exec_time_ns"] = res.exec_time_ns
    LAST_RESULT["results"] = res.results

    outp = np.empty((B, C, H, W), np.float32)
    for k in range(NBLK):
        outp[:, k * BS:(k + 1) * BS] = \
            res.results[k]["out"].astype(np.float32).transpose(0, 1, 3, 2)
    return outp


# revision 17
# speedup vs baseline: 1.2277x; 1.2277x over previous
"""AFNO2D (channel-first) Trainium2 kernel, v3.

out = x + irfft2( softshrink(mlp2(leaky(mlp1(rfft2(x))))) * rfft2(x) )
with block-diagonal complex MLPs over 8 channel blocks of 96.

Sharding: block-parallel - core k owns spectral block k (96 channels), zero
collectives.

v3 design vs v2 (593us):
 - S2 and mix2T use fp8e4 DoubleRow matmuls: each DR instruction computes the
   complex-accumulate PAIR (two K=128/97 products summed) at 0.5 cyc/out-col,
   a 4x PE cut on those stages.  u1 and o1 are staged fp8; scale factors are
   folded into host constants to keep every fp8 tensor in range:
     fw has no 1/n -> u1 = DFT_W(x), sigma~8
     s2pair = twiddles/4  -> utr = 32*xf (bf16, max ~160 < 240 fp8 if it were)
     w1 consts /32 (bias row unscaled) -> o1 = leaky(W1 xf + b1) exact
     w2pair = 32*W2 (bias 32*b2), lambda' = 0.32 -> sh = 32*sh_ref
     ybig = sh (.) utT = 1024*y_ref (bf16), fic/fis /1024 -> exact out
 - shrink restructured: ACT drains mix2-psum to bf16 o2t once; clamp and
   subtract then run on DVE in 4x perf mode (all-SBUF bf16) instead of two
   1x psum-source passes.
 - gate arithmetic batched at nq=4 (half the instruction count + semaphores);
   ta/ybig-re on DVE (4x), tb/ybig-im on GpSimd as before.
 - utr drain pinned to ACT, vdrain alternates ACT/DVE, s1 drain ACT (fp8 out).
 - ones rows of utr/o1 memset once at start (tiles persist across batches).

PSUM rule: matmul start=True clears has_written for the WHOLE bank, so each
bank gets exactly one start=True (its first matmul); all later matmuls use
start=False (fresh ranges overwrite, accumulation ranges add).

Hardcoded shapes: x [4,768,128,128] f32, w1/w2 [2,8,96,96], b1/b2 [2,8,96].
"""

import os
import numpy as np
import ml_dtypes

B, C, H, W = 4, 768, 128, 128
NBLK, BS = 8, 96          # spectral blocks, channels per core
WF = 65                   # rfft size along W
LAM = 0.01                # softshrink threshold
NS = 0.1                  # leaky relu negative slope

SH_SCALE = 32.0           # sh = 32*sh_ref ; lambda' = 32*lam
U_SCALE = 32.0            # utr/utT = 32*xf
LAMS = LAM * SH_SCALE

BF16 = ml_dtypes.bfloat16
FP8 = ml_dtypes.float8_e4m3

LAST_RESULT = {}          # diagnostics (exec_time_ns) for the test harness


def _twiddles():
    n = 128
    wv = np.arange(n)[:, None].astype(np.float64)
    jv = np.arange(n)[None, :].astype(np.float64)
    ang = 2.0 * np.pi * wv * jv / n  # [128,128]

    # S1 DoubleRow moving [64, 2, 130]: pair = (w 0..63, w 64..127) halves of
    # the K contraction; cols 0..64 cos ; cols 65..129 -sin (imag cols 65 and
    # 129 i.e. wf=0,64 are exactly zero).  No 1/n: u1 is the unnormalized
    # DFT_W so the fp8 staging sits at sigma~8.
    fw = np.zeros((n, 130), np.float64)
    fw[:, :WF] = np.cos(ang[:, :WF])
    fw[:, WF + 1:WF + 64] = -np.sin(ang[:, 1:64])
    fwpair = np.stack([fw[0:64], fw[64:128]], axis=1)  # [64, 2, 130]

    c = np.cos(ang)
    s = np.sin(ang)
    # S2 movings (fp8, non-DR): /4 so utr = DFT2(x)/4 = 32*xf.
    s2cs = np.hstack([c, -s]) / 4.0
    s2sc = np.hstack([s, c]) / 4.0
    mcs = np.hstack([c, s])       # iDFT moving for Yr: [cos | sin]
    msc = np.hstack([-s, c])      # iDFT moving for Yi: [-sin | cos]

    # final irfft_W stationary pair (fp8 DoubleRow): plane0 = fic (x vst-re),
    # plane1 = fis (x vst-im).  vst is drained with scale 1/1024 to undo
    # ybig = 1024*y_ref, so these keep their original 1/n-only scale.
    alpha = np.full((WF, 1), 2.0)
    alpha[0, 0] = 1.0
    alpha[64, 0] = 1.0
    fic = alpha * np.cos(2.0 * np.pi * np.arange(WF)[:, None]
                         * np.arange(n)[None, :] / n) / n
    fis = np.zeros((WF, n), np.float64)                 # rows 0,64 (wf) = 0
    fis[1:64] = -2.0 * np.sin(2.0 * np.pi * np.arange(1, 64)[:, None]
                              * np.arange(n)[None, :] / n) / n
    fpair = np.stack([fic, fis], axis=1)                # [65, 2, 128]

    return (fwpair.astype(FP8), s2cs.astype(BF16), s2sc.astype(BF16),
            mcs.astype(BF16), msc.astype(BF16), fpair.astype(FP8))


def _build():
    import concourse.mybir as mybir
    import concourse.tile as tile
    import concourse.bass as _bass
    from concourse import bacc

    dt = mybir.dt
    AF = mybir.ActivationFunctionType
    ALU = mybir.AluOpType
    DR = mybir.MatmulPerfMode.DoubleRow

    nc = bacc.Bacc("TRN2", target_bir_lowering=False, debug=False)

    xt = nc.declare_dram_parameter("xt", [B, W, BS, H], dt.bfloat16, isOutput=False)
    # fp8 copy of x for the S1 DoubleRow stationaries: [b, w mod 64, w div 64,
    # c, h] so the K-half pair lives in the free dims
    xt8 = nc.declare_dram_parameter("xt8", [B, 64, 2, BS, H], dt.float8e4, isOutput=False)
    # out is [b, c, w, h]; host transposes the last two axes back
    out = nc.declare_dram_parameter("out", [B, BS, W, H], dt.bfloat16, isOutput=True)

    fwp_d = nc.declare_dram_parameter("fwpair", [64, 2, 130], dt.float8e4, isOutput=False)
    s2cs_d = nc.declare_dram_parameter("s2cs", [128, 256], dt.bfloat16, isOutput=False)
    s2sc_d = nc.declare_dram_parameter("s2sc", [128, 256], dt.bfloat16, isOutput=False)
    mcs_d = nc.declare_dram_parameter("mcs", [128, 256], dt.bfloat16, isOutput=False)
    msc_d = nc.declare_dram_parameter("msc", [128, 256], dt.bfloat16, isOutput=False)
    fp_d = nc.declare_dram_parameter("fpair", [WF, 2, 128], dt.float8e4, isOutput=False)

    wnames = ("w1ra", "w1ia", "w1in", "w1r_")
    wshapes = {"w1ra": [BS + 1, 128], "w1ia": [BS + 1, 128],
               "w1in": [BS, 128], "w1r_": [BS, 128]}
    wds = {nm: nc.declare_dram_parameter(nm, wshapes[nm], dt.bfloat16,
                                         isOutput=False)
           for nm in wnames}
    w2p_d = nc.declare_dram_parameter("w2pair", [BS + 1, 2, 192], dt.float8e4,
                                      isOutput=False)

    dbg = bool(int(os.environ.get("AFNO_DEBUG", "0")))
    if dbg:
        d_u1 = nc.declare_dram_parameter("d_u1", [128, BS, 130], dt.bfloat16, isOutput=True)
        d_utr = nc.declare_dram_parameter("d_utr", [BS + 1, 2, WF, 128], dt.bfloat16, isOutput=True)
        d_utT = nc.declare_dram_parameter("d_utT", [128, 2, WF, BS], dt.bfloat16, isOutput=True)
        d_o1 = nc.declare_dram_parameter("d_o1", [BS + 1, 2, 512], dt.float8e4, isOutput=True)
        d_o2 = nc.declare_dram_parameter("d_o2", [128, 4, 192], dt.bfloat16, isOutput=True)
        d_ybig = nc.declare_dram_parameter("d_ybig", [128, 2, WF, BS], dt.bfloat16, isOutput=True)
        d_vst = nc.declare_dram_parameter("d_vst", [WF, 2, 4, 128], dt.float8e4, isOutput=True)

    with tile.TileContext(nc) as tc:
        with (
            tc.tile_pool(name="consts", bufs=1) as consts,
            tc.tile_pool(name="xt8", bufs=2) as xt8_p,
            tc.tile_pool(name="rx", bufs=3) as rx_p,
            tc.tile_pool(name="u1", bufs=1) as u1_p,
            tc.tile_pool(name="utr", bufs=1) as utr_p,
            tc.tile_pool(name="utT", bufs=1) as utT_p,
            tc.tile_pool(name="ybig", bufs=1) as ybig_p,
            tc.tile_pool(name="o1", bufs=3) as o1_p,
            tc.tile_pool(name="o2t", bufs=2) as o2t_p,
            tc.tile_pool(name="sg", bufs=3) as sg_p,
            tc.tile_pool(name="vst", bufs=2) as vst_p,
            tc.tile_pool(name="ot", bufs=3) as ot_p,
            tc.tile_pool(name="pS", bufs=2, space="PSUM") as pS,
            tc.tile_pool(name="pM", bufs=2, space="PSUM") as pM,
            tc.tile_pool(name="pN", bufs=2, space="PSUM") as pN,
        ):
            # ---------------- constants ----------------
            def cload(dparam, shape, tag, dtype=dt.bfloat16):
                t = consts.tile(shape, dtype, tag=tag, name=tag)
                nc.sync.dma_start(out=t[:], in_=dparam[:])
                return t

            fwpair = cload(fwp_d, [64, 2, 130], "fwpair", dt.float8e4)
            s2cs = cload(s2cs_d, [128, 256], "s2cs")
            s2sc = cload(s2sc_d, [128, 256], "s2sc")
            mcs = cload(mcs_d, [128, 256], "mcs")
            msc = cload(msc_d, [128, 256], "msc")
            fpair = cload(fp_d, [WF, 2, 128], "fpair", dt.float8e4)
            wt = {nm: cload(wds[nm], wshapes[nm], nm) for nm in wnames}
            w2pair = cload(w2p_d, [BS + 1, 2, 192], "w2pair", dt.float8e4)

            # persistent tiles: ones/zero rows set once (pools rotate over
            # fixed buffers; drains never touch these rows)
            utr_t = utr_p.tile([BS + 1, 2, WF, 128], dt.bfloat16, tag="utr")
            nc.gpsimd.memset(utr_t[BS:BS + 1, 0, :, :], 1.0)
            nc.gpsimd.memset(utr_t[BS:BS + 1, 1, :, :], 0.0)
            o1_pre = [o1_p.tile([BS + 1, 2, 512], dt.float8e4, tag="o1",
                                name="o1") for _ in range(3)]
            for t in o1_pre:
                nc.gpsimd.memset(t[BS:BS + 1, 0, :], 1.0)
                nc.gpsimd.memset(t[BS:BS + 1, 1, :], 0.0)

            def load_x(b):
                t8 = xt8_p.tile([64, 2, BS, 128], dt.float8e4, tag="xt8",
                                name="xt8")
                nc.sync.dma_start(out=t8[:], in_=xt8[b, :, :, :, :])
                return t8

            def s1(b, t8):
                # rfft-W: one fp8 DoubleRow matmul per channel, pair = the
                # two K-halves (w 0..63 / 64..127).
                u1 = u1_p.tile([128, BS, 130], dt.bfloat16, tag="u1")
                x8 = t8[:, :, :, :]
                for g in range(BS // 3):
                    ps = pS.tile([128, 3, 130], dt.float32, tag="pS", name="ps1")
                    for k in range(3):
                        c = 3 * g + k
                        lhsT = _bass.AP(tensor=x8.tensor,
                                        offset=x8.offset + 128 * c,
                                        ap=[x8.ap[0], [BS * 128, 2], [1, 128]])
                        nc.tensor.matmul(ps[:, k, :], lhsT, fwpair[:, :, :],
                                         start=(k == 0), stop=(k == 2),
                                         perf_mode=DR, skip_group_check=True)
                    nc.scalar.activation(u1[:, 3 * g:3 * g + 3, :], ps[:, :, :],
                                         AF.Copy, bias=0.0, scale=1.0)
                if dbg and b == 0:
                    nc.sync.dma_start(out=d_u1[:, :, :], in_=u1[:, :, :])
                return u1

            def s2(b, u1):
                # DFT along H, data-stationary per frequency (fp8 operands,
                # plain matmuls); utr is fp8-free (bf16) since mix1 is bf16.
                utr = utr_p.tile([BS + 1, 2, WF, 128], dt.bfloat16, tag="utr")
                utT = utT_p.tile([128, 2, WF, BS], dt.bfloat16, tag="utT")
                xbar_after = {11: (0, 24), 23: (24, 48), 32: (48, WF)}
                for jp in range(33):
                    js = [j for j in (2 * jp, 2 * jp + 1) if j < WF]
                    ps = pS.tile([128, 2, 256], dt.float32, tag="pS", name="ps2")
                    for q, j in enumerate(js):
                        lr = u1[:, :, j]
                        li = u1[:, :, 65 + j]
                        nc.tensor.matmul(ps[0:BS, q, :], lr, s2cs,
                                         start=(q == 0), stop=False,
                                         skip_group_check=True)
                        nc.tensor.matmul(ps[0:BS, q, :], li, s2sc,
                                         start=False, stop=(q == len(js) - 1),
                                         skip_group_check=True)
                    src = ps[0:BS, 0:len(js), :].rearrange(
                        "p q (r h) -> p q r h", r=2)
                    dst = utr[0:BS, :, js[0]:js[0] + len(js), :].rearrange(
                        "p r j h -> p j r h")
                    if jp % 2 == 0:
                        nc.scalar.activation(dst, src, AF.Copy, bias=0.0,
                                             scale=1.0)
                    else:
                        nc.vector.tensor_copy(dst, src)
                    if jp in xbar_after:
                        lo, hi = xbar_after[jp]
                        for ri in range(2):
                            nc.sync.dma_start_transpose(
                                out=utT[:, ri, lo:hi, :],
                                in_=utr[0:BS, ri, lo:hi, :])
                if dbg and b == 0:
                    nc.sync.dma_start(out=d_utr[:, :, :, :], in_=utr[:, :, :, :])
                    nc.sync.dma_start(out=d_utT[:, :, :, :], in_=utT[:, :, :, :])
                return utr, utT

            def mix(b, utr, utT):
                ybig = ybig_p.tile([128, 2, WF, BS], dt.bfloat16, tag="ybig")
                nchunk = 17                   # 16 x 512 + 1 x 128

                def mix1(ci):
                    j0 = 4 * ci
                    njj = min(4, WF - j0)
                    sz = njj * 128
                    ur = utr[0:BS + 1, 0, j0:j0 + njj, :]
                    ui = utr[0:BS, 1, j0:j0 + njj, :]
                    p1 = pM.tile([128, 1024], dt.float32, tag="pM", name="p1")
                    nc.tensor.matmul(p1[:, 0:sz], wt["w1ra"], ur,
                                     start=True, stop=False, skip_group_check=True)
                    nc.tensor.matmul(p1[:, 512:512 + sz], wt["w1ia"], ur,
                                     start=True, stop=False, skip_group_check=True)
                    nc.tensor.matmul(p1[:, 0:sz], wt["w1in"], ui,
                                     start=False, stop=False, skip_group_check=True)
                    nc.tensor.matmul(p1[:, 512:512 + sz], wt["w1r_"], ui,
                                     start=False, stop=True, skip_group_check=True)
                    o1 = o1_p.tile([BS + 1, 2, 512], dt.float8e4, tag="o1",
                                   name="o1")
                    psrc = p1[0:BS, :].rearrange("p (r f) -> p r f", r=2)[:, :, 0:sz]
                    nc.scalar.activation(o1[0:BS, :, 0:sz], psrc, AF.Prelu,
                                         bias=0.0, scale=1.0, alpha=NS)
                    return (ci, j0, njj, o1)

                def mix2T(st):
                    ci, j0, njj, o1 = st
                    if dbg and b == 0 and ci == 0:
                        nc.sync.dma_start(out=d_o1[:, :, :], in_=o1[:, :, :])
                    o2t = o2t_p.tile([128, 4, 192], dt.bfloat16, tag="o2t",
                                     name="o2t")
                    o1a = o1[:, :, :]
                    for t0 in range(0, njj, 2):
                        js = [jj for jj in (t0, t0 + 1) if jj < njj]
                        p2 = pN.tile([128, 2, 192], dt.float32, tag="pN",
                                     name="p2")
                        for q, jj in enumerate(js):
                            lhsT = _bass.AP(
                                tensor=o1a.tensor, offset=o1a.offset + 128 * jj,
                                ap=[o1a.ap[0], [512, 2], [1, 128]])
                            nc.tensor.matmul(p2[:, q, :], lhsT, w2pair[:, :, :],
                                             start=(q == 0), stop=(q == len(js) - 1),
                                             perf_mode=DR, skip_group_check=True)
                        nq = len(js)
                        nc.scalar.activation(o2t[:, t0:t0 + nq, :], p2[:, 0:nq, :],
                                             AF.Copy, bias=0.0, scale=1.0)
                    if dbg and b == 0 and ci == 0:
                        nc.sync.dma_start(out=d_o2[:, :, :], in_=o2t[:, :, :])
                    # softshrink on bf16 SBUF (DVE 4x): sh = o2t - clamp(o2t)
                    nq = njj
                    cl = sg_p.tile([128, 4, 192], dt.bfloat16, tag="cl",
                                   name="cl")
                    nc.vector.tensor_scalar(cl[:, 0:nq, :], o2t[:, 0:nq, :],
                                            -LAMS, LAMS, ALU.max, ALU.min)
                    sh = sg_p.tile([128, 4, 192], dt.bfloat16, tag="sh",
                                   name="sh")
                    nc.vector.tensor_sub(sh[:, 0:nq, :], o2t[:, 0:nq, :],
                                         cl[:, 0:nq, :])
                    # gate: y = s * u (complex), in [hf, c] layout, nq j's at once
                    _s = sh[:, :, :]
                    sr2 = _bass.AP(
                        tensor=_s.tensor, offset=_s.offset,
                        ap=[_s.ap[0], [192, nq], [0, 2], [1, BS]])
                    si2 = _bass.AP(
                        tensor=_s.tensor, offset=_s.offset + 96,
                        ap=[_s.ap[0], [192, nq], [0, 2], [1, BS]])
                    u2 = utT[:, :, j0:j0 + nq, :].rearrange(
                        "p r j c -> p j r c")
                    ta = sg_p.tile([128, 4, 2, BS], dt.bfloat16, tag="ta",
                                   name="ta")
                    tb = sg_p.tile([128, 4, 2, BS], dt.bfloat16, tag="tb",
                                   name="tb")
                    nc.vector.tensor_mul(ta[:, 0:nq, :, :], sr2, u2)
                    nc.gpsimd.tensor_mul(tb[:, 0:nq, :, :], si2, u2)
                    nc.vector.tensor_sub(
                        ybig[:, 0, j0:j0 + nq, :],
                        ta[:, 0:nq, 0, :], tb[:, 0:nq, 1, :])
                    nc.gpsimd.tensor_add(
                        ybig[:, 1, j0:j0 + nq, :],
                        ta[:, 0:nq, 1, :], tb[:, 0:nq, 0, :])

                prev = None
                for ci in range(nchunk):
                    cur = mix1(ci)
                    if prev is not None:
                        mix2T(prev)
                    prev = cur
                mix2T(prev)
                if dbg and b == 0:
                    nc.sync.dma_start(out=d_ybig[:, :, :, :], in_=ybig[:, :, :, :])
                return ybig

            def idft_final(b, ybig):
                def idftA(p):
                    pV = pN.tile([128, 2, 256], dt.float32, tag="pN", name="pV")
                    for cc in range(2):
                        c = 2 * p + cc
                        yr = ybig[:, 0, :, c]
                        yi = ybig[:, 1, :, c]
                        nc.tensor.matmul(pV[0:WF, cc, :], yr, mcs,
                                         start=(cc == 0), stop=False,
                                         skip_group_check=True)
                        nc.tensor.matmul(pV[0:WF, cc, :], yi, msc,
                                         start=False, stop=(cc == 1),
                                         skip_group_check=True)
                    return pV

                def vdrain(p, pV, vst):
                    # vst [65, 2(re/im), 4(c), 128(h)] fp8, scaled 1/1024 to
                    # undo ybig = 1024*y_ref; engine rotates ACT/DVE/Pool.
                    q = p % 2
                    dst = vst[0:WF, :, 2 * q:2 * q + 2, :]
                    src = pV[0:WF, :, :].rearrange("p c (r h) -> p r c h", r=2)
                    if p % 2 == 0:
                        nc.scalar.activation(dst, src, AF.Copy, bias=0.0,
                                             scale=1.0 / 1024.0)
                    else:
                        nc.vector.tensor_scalar(dst, src, 1.0 / 1024.0, None,
                                                ALU.mult)

                def final(qd, vst, xh4):
                    pO = pM.tile([128, 4, 128], dt.float32, tag="pM", name="pO")
                    mov = vst[:, :, :, :].rearrange("p r c h -> p r (c h)")
                    nc.tensor.matmul(pO[:, :, :].rearrange("p c f -> p (c f)"),
                                     fpair[:, :, :], mov,
                                     start=True, stop=True,
                                     perf_mode=DR, skip_group_check=True)
                    c0 = 4 * qd
                    ot = ot_p.tile([128, 4, 128], dt.bfloat16, tag="ot",
                                   name="ot")
                    nc.vector.tensor_add(
                        ot[:, :, :].rearrange("p c f -> p (c f)"),
                        pO[:, :, :].rearrange("p c f -> p (c f)"),
                        xh4.rearrange("p c f -> p (c f)"))
                    nc.sync.dma_start(
                        out=out[b, c0:c0 + 4, :, :].rearrange("c w h -> w c h"),
                        in_=ot[:, :, :])

                vsts = {}
                rxs = {}
                for p in range(BS // 2):
                    qd = p // 2
                    if p % 2 == 0:
                        vsts[qd] = vst_p.tile([WF, 2, 4, 128], dt.float8e4,
                                              tag="vst", name="vst")
                        # prefetch the residual x slab for this 4-channel group
                        rxs[qd] = rx_p.tile([128, 4, 128], dt.bfloat16,
                                            tag="rx", name="rx")
                        nc.sync.dma_start(out=rxs[qd][:],
                                          in_=xt[b, :, 4 * qd:4 * qd + 4, :])
                    pV = idftA(p)
                    vdrain(p, pV, vsts[qd])
                    if dbg and b == 0 and p == 1:
                        nc.sync.dma_start(out=d_vst[:, :, :, :], in_=vsts[0][:, :, :, :])
                    if p % 2 == 1 and p >= 3:
                        final(qd - 1, vsts[qd - 1], rxs.pop(qd - 1))
                final(BS // 4 - 1, vsts[BS // 4 - 1], rxs.pop(BS // 4 - 1))

            # ---- batch-level software pipeline: S1(b+1) fills the PE while
            # ---- the mix(b) elementwise tail drains; S2(b+1) follows idft(b).
            t8_all = {0: load_x(0)}
            u1 = s1(0, t8_all[0])
            utr, utT = s2(0, u1)
            for b in range(B):
                if b + 1 < B:
                    t8_all[b + 1] = load_x(b + 1)
                ybig = mix(b, utr, utT)
                if b + 1 < B:
                    u1 = s1(b + 1, t8_all[b + 1])
                idft_final(b, ybig)
                del t8_all[b]
                if b + 1 < B:
                    utr, utT = s2(b + 1, u1)

    nc.finalize()
    return nc


_BUILT = None


def _get_built():
    global _BUILT
    if _BUILT is None:
        _BUILT = _build()
    return _BUILT


def _make_in_maps(x, w1, b1, w2, b2):
    fwpair, s2cs, s2sc, mcs, msc, fpair = _twiddles()
    in_maps = []
    for k in range(NBLK):
        xs = x[:, k * BS:(k + 1) * BS]
        w1r, w1i = w1[0, k], w1[1, k]
        w2r, w2i = w2[0, k], w2[1, k]

        def pad128(a):
            o = np.zeros((a.shape[0], 128), np.float32)
            o[:, 0:BS] = a
            return o.astype(BF16)

        # mix1 consumes utr = 32*xf: weights /32, bias row unscaled
        inv = 1.0 / U_SCALE
        # mix2 DoubleRow pair: plane0 x o1-re, plane1 x o1-im; x32 so
        # sh = 32*sh_ref and the fp8 weights sit at ~0.6
        w2p0 = np.vstack([np.hstack([w2r, w2i]),
                          np.hstack([b2[0, k][None, :], b2[1, k][None, :]])])
        w2p1 = np.vstack([np.hstack([-w2i, w2r]), np.zeros((1, 192))])
        w2pair = (SH_SCALE * np.stack([w2p0, w2p1], axis=1)).astype(FP8)
        xsw = np.ascontiguousarray(xs.transpose(0, 3, 1, 2))  # [B, W, BS, H]
        m = {
            "xt": xsw.astype(BF16),
            "xt8": xsw.reshape(B, 2, 64, BS, 128).transpose(0, 2, 1, 3, 4
                                                            ).astype(FP8),
            "fwpair": fwpair, "s2cs": s2cs, "s2sc": s2sc,
            "mcs": mcs, "msc": msc, "fpair": fpair,
            "w1ra": pad128(np.vstack([inv * w1r, b1[0, k][None, :]])),
            "w1ia": pad128(np.vstack([inv * w1i, b1[1, k][None, :]])),
            "w1in": pad128(-inv * w1i), "w1r_": pad128(inv * w1r),
            "w2pair": w2pair,
        }
        in_maps.append(m)
    return in_maps


def kernel(x, w1, b1, w2, b2):
    from concourse.bass_utils import run_bass_kernel_spmd

    nc = _get_built()
    in_maps = _make_in_maps(x, w1, b1, w2, b2)

    trace = bool(int(os.environ.get("AFNO_TRACE", "0")))
    kw = {}
    if trace:
        import tempfile
        kw["tmpdir"] = tempfile.mkdtemp(prefix="afno_trace_")
        LAST_RESULT["trace_dir"] = kw["tmpdir"]
    res = run_bass_kernel_spmd(nc, in_maps, core_ids=list(range(NBLK)),
                               trace=trace, **kw)
    LAST_RESULT["exec_time_ns"] = res.exec_time_ns
    LAST_RESULT["results"] = res.results

    outp = np.empty((B, C, H, W), np.float32)
    for k in range(NBLK):
        outp[:, k * BS:(k + 1) * BS] = \
            res.results[k]["out"].astype(np.float32).transpose(0, 1, 3, 2)
    return outp


# revision 20
# speedup vs baseline: 1.3296x; 1.0830x over previous
"""AFNO2D (channel-first) Trainium2 kernel, v3.

out = x + irfft2( softshrink(mlp2(leaky(mlp1(rfft2(x))))) * rfft2(x) )
with block-diagonal complex MLPs over 8 channel blocks of 96.

Sharding: block-parallel - core k owns spectral block k (96 channels), zero
collectives.

v3 design vs v2 (593us):
 - S2 and mix2T use fp8e4 DoubleRow matmuls: each DR instruction computes the
   complex-accumulate PAIR (two K=128/97 products summed) at 0.5 cyc/out-col,
   a 4x PE cut on those stages.  u1 and o1 are staged fp8; scale factors are
   folded into host constants to keep every fp8 tensor in range:
     fw has no 1/n -> u1 = DFT_W(x), sigma~8
     s2pair = twiddles/4  -> utr = 32*xf (bf16, max ~160 < 240 fp8 if it were)
     w1 consts /32 (bias row unscaled) -> o1 = leaky(W1 xf + b1) exact
     w2pair = 32*W2 (bias 32*b2), lambda' = 0.32 -> sh = 32*sh_ref
     ybig = sh (.) utT = 1024*y_ref (bf16), fic/fis /1024 -> exact out
 - shrink restructured: ACT drains mix2-psum to bf16 o2t once; clamp and
   subtract then run on DVE in 4x perf mode (all-SBUF bf16) instead of two
   1x psum-source passes.
 - gate arithmetic batched at nq=4 (half the instruction count + semaphores);
   ta/ybig-re on DVE (4x), tb/ybig-im on GpSimd as before.
 - utr drain pinned to ACT, vdrain alternates ACT/DVE, s1 drain ACT (fp8 out).
 - ones rows of utr/o1 memset once at start (tiles persist across batches).

PSUM rule: matmul start=True clears has_written for the WHOLE bank, so each
bank gets exactly one start=True (its first matmul); all later matmuls use
start=False (fresh ranges overwrite, accumulation ranges add).

Hardcoded shapes: x [4,768,128,128] f32, w1/w2 [2,8,96,96], b1/b2 [2,8,96].
"""

import os
import numpy as np
import ml_dtypes

B, C, H, W = 4, 768, 128, 128
NBLK, BS = 8, 96          # spectral blocks, channels per core
WF = 65                   # rfft size along W
LAM = 0.01                # softshrink threshold
NS = 0.1                  # leaky relu negative slope

SH_SCALE = 32.0           # sh = 32*sh_ref ; lambda' = 32*lam
U_SCALE = 32.0            # utr/utT = 32*xf
LAMS = LAM * SH_SCALE

BF16 = ml_dtypes.bfloat16
FP8 = ml_dtypes.float8_e4m3

LAST_RESULT = {}          # diagnostics (exec_time_ns) for the test harness


def _twiddles():
    n = 128
    wv = np.arange(n)[:, None].astype(np.float64)
    jv = np.arange(n)[None, :].astype(np.float64)
    ang = 2.0 * np.pi * wv * jv / n  # [128,128]

    # S1 DoubleRow moving [64, 2, 130]: pair = (w 0..63, w 64..127) halves of
    # the K contraction; cols 0..64 cos ; cols 65..129 -sin (imag cols 65 and
    # 129 i.e. wf=0,64 are exactly zero).  No 1/n: u1 is the unnormalized
    # DFT_W so the fp8 staging sits at sigma~8.
    fw = np.zeros((n, 130), np.float64)
    fw[:, :WF] = np.cos(ang[:, :WF])
    fw[:, WF + 1:WF + 64] = -np.sin(ang[:, 1:64])
    fwpair = np.stack([fw[0:64], fw[64:128]], axis=1)  # [64, 2, 130]

    c = np.cos(ang)
    s = np.sin(ang)
    # S2 movings (fp8, non-DR): /4 so utr = DFT2(x)/4 = 32*xf.
    s2cs = np.hstack([c, -s]) / 4.0
    s2sc = np.hstack([s, c]) / 4.0
    mcs = np.hstack([c, s])       # iDFT moving for Yr: [cos | sin]
    msc = np.hstack([-s, c])      # iDFT moving for Yi: [-sin | cos]

    # final irfft_W stationary pair (fp8 DoubleRow): plane0 = fic (x vst-re),
    # plane1 = fis (x vst-im).  vst is drained with scale 1/1024 to undo
    # ybig = 1024*y_ref, so these keep their original 1/n-only scale.
    alpha = np.full((WF, 1), 2.0)
    alpha[0, 0] = 1.0
    alpha[64, 0] = 1.0
    fic = alpha * np.cos(2.0 * np.pi * np.arange(WF)[:, None]
                         * np.arange(n)[None, :] / n) / n
    fis = np.zeros((WF, n), np.float64)                 # rows 0,64 (wf) = 0
    fis[1:64] = -2.0 * np.sin(2.0 * np.pi * np.arange(1, 64)[:, None]
                              * np.arange(n)[None, :] / n) / n
    fpair = np.stack([fic, fis], axis=1)                # [65, 2, 128]

    return (fwpair.astype(FP8), s2cs.astype(BF16), s2sc.astype(BF16),
            mcs.astype(BF16), msc.astype(BF16), fpair.astype(FP8))


def _build():
    import concourse.mybir as mybir
    import concourse.tile as tile
    import concourse.bass as _bass
    from concourse import bacc

    dt = mybir.dt
    AF = mybir.ActivationFunctionType
    ALU = mybir.AluOpType
    DR = mybir.MatmulPerfMode.DoubleRow

    nc = bacc.Bacc("TRN2", target_bir_lowering=False, debug=False)

    xt = nc.declare_dram_parameter("xt", [B, W, BS, H], dt.bfloat16, isOutput=False)
    # fp8 copy of x for the S1 DoubleRow stationaries: [b, w mod 64, w div 64,
    # c, h] so the K-half pair lives in the free dims
    xt8 = nc.declare_dram_parameter("xt8", [B, 64, 2, BS, H], dt.float8e4, isOutput=False)
    # out is [b, c, w, h]; host transposes the last two axes back
    out = nc.declare_dram_parameter("out", [B, BS, W, H], dt.bfloat16, isOutput=True)

    fwp_d = nc.declare_dram_parameter("fwpair", [64, 2, 130], dt.float8e4, isOutput=False)
    s2cs_d = nc.declare_dram_parameter("s2cs", [128, 256], dt.bfloat16, isOutput=False)
    s2sc_d = nc.declare_dram_parameter("s2sc", [128, 256], dt.bfloat16, isOutput=False)
    mcs_d = nc.declare_dram_parameter("mcs", [128, 256], dt.bfloat16, isOutput=False)
    msc_d = nc.declare_dram_parameter("msc", [128, 256], dt.bfloat16, isOutput=False)
    fp_d = nc.declare_dram_parameter("fpair", [WF, 2, 128], dt.float8e4, isOutput=False)

    wnames = ("w1ra", "w1ia", "w1in", "w1r_")
    wshapes = {"w1ra": [BS + 1, 128], "w1ia": [BS + 1, 128],
               "w1in": [BS, 128], "w1r_": [BS, 128]}
    wds = {nm: nc.declare_dram_parameter(nm, wshapes[nm], dt.bfloat16,
                                         isOutput=False)
           for nm in wnames}
    w2p_d = nc.declare_dram_parameter("w2pair", [BS + 1, 2, 192], dt.float8e4,
                                      isOutput=False)

    dbg = bool(int(os.environ.get("AFNO_DEBUG", "0")))
    if dbg:
        d_u1 = nc.declare_dram_parameter("d_u1", [128, BS, 130], dt.bfloat16, isOutput=True)
        d_utr = nc.declare_dram_parameter("d_utr", [BS + 1, 2, WF, 128], dt.bfloat16, isOutput=True)
        d_utT = nc.declare_dram_parameter("d_utT", [128, 2, WF, BS], dt.bfloat16, isOutput=True)
        d_o1 = nc.declare_dram_parameter("d_o1", [BS + 1, 2, 512], dt.float8e4, isOutput=True)
        d_o2 = nc.declare_dram_parameter("d_o2", [128, 4, 192], dt.bfloat16, isOutput=True)
        d_ybig = nc.declare_dram_parameter("d_ybig", [128, 2, WF, BS], dt.bfloat16, isOutput=True)
        d_vst = nc.declare_dram_parameter("d_vst", [WF, 2, 4, 128], dt.float8e4, isOutput=True)

    with tile.TileContext(nc) as tc:
        with (
            tc.tile_pool(name="consts", bufs=1) as consts,
            tc.tile_pool(name="xt8", bufs=2) as xt8_p,
            tc.tile_pool(name="rx", bufs=3) as rx_p,
            tc.tile_pool(name="u1", bufs=1) as u1_p,
            tc.tile_pool(name="utr", bufs=1) as utr_p,
            tc.tile_pool(name="utT", bufs=1) as utT_p,
            tc.tile_pool(name="ybig", bufs=1) as ybig_p,
            tc.tile_pool(name="o1", bufs=3) as o1_p,
            tc.tile_pool(name="o2t", bufs=2) as o2t_p,
            tc.tile_pool(name="sg", bufs=3) as sg_p,
            tc.tile_pool(name="vst", bufs=2) as vst_p,
            tc.tile_pool(name="ot", bufs=3) as ot_p,
            tc.tile_pool(name="pS", bufs=2, space="PSUM") as pS,
            tc.tile_pool(name="pM", bufs=2, space="PSUM") as pM,
            tc.tile_pool(name="pN", bufs=2, space="PSUM") as pN,
        ):
            # ---------------- constants ----------------
            def cload(dparam, shape, tag, dtype=dt.bfloat16):
                t = consts.tile(shape, dtype, tag=tag, name=tag)
                nc.sync.dma_start(out=t[:], in_=dparam[:])
                return t

            fwpair = cload(fwp_d, [64, 2, 130], "fwpair", dt.float8e4)
            s2cs = cload(s2cs_d, [128, 256], "s2cs")
            s2sc = cload(s2sc_d, [128, 256], "s2sc")
            mcs = cload(mcs_d, [128, 256], "mcs")
            msc = cload(msc_d, [128, 256], "msc")
            fpair = cload(fp_d, [WF, 2, 128], "fpair", dt.float8e4)
            wt = {nm: cload(wds[nm], wshapes[nm], nm) for nm in wnames}
            w2pair = cload(w2p_d, [BS + 1, 2, 192], "w2pair", dt.float8e4)

            # persistent tiles: ones/zero rows set once (pools rotate over
            # fixed buffers; drains never touch these rows)
            utr_t = utr_p.tile([BS + 1, 2, WF, 128], dt.bfloat16, tag="utr")
            nc.gpsimd.memset(utr_t[BS:BS + 1, 0, :, :], 1.0)
            nc.gpsimd.memset(utr_t[BS:BS + 1, 1, :, :], 0.0)
            o1_pre = [o1_p.tile([BS + 1, 2, 512], dt.float8e4, tag="o1",
                                name="o1") for _ in range(3)]
            for t in o1_pre:
                nc.gpsimd.memset(t[BS:BS + 1, 0, :], 1.0)
                nc.gpsimd.memset(t[BS:BS + 1, 1, :], 0.0)

            def load_x(b):
                t8 = xt8_p.tile([64, 2, BS, 128], dt.float8e4, tag="xt8",
                                name="xt8")
                nc.sync.dma_start(out=t8[:], in_=xt8[b, :, :, :, :])
                return t8

            def s1(b, t8, box):
                # rfft-W: one fp8 DoubleRow matmul per channel, pair = the
                # two K-halves (w 0..63 / 64..127).  Generator: yields after
                # each psum group so the caller can weave it into another
                # phase's emission stream.
                u1 = u1_p.tile([128, BS, 130], dt.bfloat16, tag="u1")
                box["u1"] = u1
                x8 = t8[:, :, :, :]
                for g in range(BS // 3):
                    ps = pS.tile([128, 3, 130], dt.float32, tag="pS", name="ps1")
                    for k in range(3):
                        c = 3 * g + k
                        lhsT = _bass.AP(tensor=x8.tensor,
                                        offset=x8.offset + 128 * c,
                                        ap=[x8.ap[0], [BS * 128, 2], [1, 128]])
                        nc.tensor.matmul(ps[:, k, :], lhsT, fwpair[:, :, :],
                                         start=(k == 0), stop=(k == 2),
                                         perf_mode=DR, skip_group_check=True)
                    nc.scalar.activation(u1[:, 3 * g:3 * g + 3, :], ps[:, :, :],
                                         AF.Copy, bias=0.0, scale=1.0)
                    yield
                if dbg and b == 0:
                    nc.sync.dma_start(out=d_u1[:, :, :], in_=u1[:, :, :])

            def s2(b, u1, box):
                # DFT along H, data-stationary per frequency.  Generator:
                # yields after each jp pair.
                utr = utr_p.tile([BS + 1, 2, WF, 128], dt.bfloat16, tag="utr")
                utT = utT_p.tile([128, 2, WF, BS], dt.bfloat16, tag="utT")
                box["utr"], box["utT"] = utr, utT
                xbar_after = {11: (0, 24), 23: (24, 48), 32: (48, WF)}
                for jp in range(33):
                    js = [j for j in (2 * jp, 2 * jp + 1) if j < WF]
                    ps = pS.tile([128, 2, 256], dt.float32, tag="pS", name="ps2")
                    for q, j in enumerate(js):
                        lr = u1[:, :, j]
                        li = u1[:, :, 65 + j]
                        nc.tensor.matmul(ps[0:BS, q, :], lr, s2cs,
                                         start=(q == 0), stop=False,
                                         skip_group_check=True)
                        nc.tensor.matmul(ps[0:BS, q, :], li, s2sc,
                                         start=False, stop=(q == len(js) - 1),
                                         skip_group_check=True)
                    src = ps[0:BS, 0:len(js), :].rearrange(
                        "p q (r h) -> p q r h", r=2)
                    dst = utr[0:BS, :, js[0]:js[0] + len(js), :].rearrange(
                        "p r j h -> p j r h")
                    if jp % 2 == 0:
                        nc.scalar.activation(dst, src, AF.Copy, bias=0.0,
                                             scale=1.0)
                    else:
                        nc.vector.tensor_copy(dst, src)
                    if jp in xbar_after:
                        lo, hi = xbar_after[jp]
                        for ri in range(2):
                            nc.sync.dma_start_transpose(
                                out=utT[:, ri, lo:hi, :],
                                in_=utr[0:BS, ri, lo:hi, :])
                    yield
                if dbg and b == 0:
                    nc.sync.dma_start(out=d_utr[:, :, :, :], in_=utr[:, :, :, :])
                    nc.sync.dma_start(out=d_utT[:, :, :, :], in_=utT[:, :, :, :])

            def mix(b, utr, utT, weave=None):
                ybig = ybig_p.tile([128, 2, WF, BS], dt.bfloat16, tag="ybig")
                nchunk = 17                   # 16 x 512 + 1 x 128

                def mix1(ci):
                    j0 = 4 * ci
                    njj = min(4, WF - j0)
                    sz = njj * 128
                    ur = utr[0:BS + 1, 0, j0:j0 + njj, :]
                    ui = utr[0:BS, 1, j0:j0 + njj, :]
                    p1 = pM.tile([128, 1024], dt.float32, tag="pM", name="p1")
                    nc.tensor.matmul(p1[:, 0:sz], wt["w1ra"], ur,
                                     start=True, stop=False, skip_group_check=True)
                    nc.tensor.matmul(p1[:, 512:512 + sz], wt["w1ia"], ur,
                                     start=True, stop=False, skip_group_check=True)
                    nc.tensor.matmul(p1[:, 0:sz], wt["w1in"], ui,
                                     start=False, stop=False, skip_group_check=True)
                    nc.tensor.matmul(p1[:, 512:512 + sz], wt["w1r_"], ui,
                                     start=False, stop=True, skip_group_check=True)
                    o1 = o1_p.tile([BS + 1, 2, 512], dt.float8e4, tag="o1",
                                   name="o1")
                    psrc = p1[0:BS, :].rearrange("p (r f) -> p r f", r=2)[:, :, 0:sz]
                    nc.scalar.activation(o1[0:BS, :, 0:sz], psrc, AF.Prelu,
                                         bias=0.0, scale=1.0, alpha=NS)
                    return (ci, j0, njj, o1)

                def mix2T(st):
                    ci, j0, njj, o1 = st
                    if dbg and b == 0 and ci == 0:
                        nc.sync.dma_start(out=d_o1[:, :, :], in_=o1[:, :, :])
                    o2t = o2t_p.tile([128, 4, 192], dt.bfloat16, tag="o2t",
                                     name="o2t")
                    o1a = o1[:, :, :]
                    for t0 in range(0, njj, 2):
                        js = [jj for jj in (t0, t0 + 1) if jj < njj]
                        p2 = pN.tile([128, 2, 192], dt.float32, tag="pN",
                                     name="p2")
                        for q, jj in enumerate(js):
                            lhsT = _bass.AP(
                                tensor=o1a.tensor, offset=o1a.offset + 128 * jj,
                                ap=[o1a.ap[0], [512, 2], [1, 128]])
                            nc.tensor.matmul(p2[:, q, :], lhsT, w2pair[:, :, :],
                                             start=(q == 0), stop=(q == len(js) - 1),
                                             perf_mode=DR, skip_group_check=True)
                        nq = len(js)
                        nc.scalar.activation(o2t[:, t0:t0 + nq, :], p2[:, 0:nq, :],
                                             AF.Copy, bias=0.0, scale=1.0)
                    if dbg and b == 0 and ci == 0:
                        nc.sync.dma_start(out=d_o2[:, :, :], in_=o2t[:, :, :])
                    # softshrink on bf16 SBUF (DVE 4x): sh = o2t - clamp(o2t)
                    nq = njj
                    cl = sg_p.tile([128, 4, 192], dt.bfloat16, tag="cl",
                                   name="cl")
                    nc.vector.tensor_scalar(cl[:, 0:nq, :], o2t[:, 0:nq, :],
                                            -LAMS, LAMS, ALU.max, ALU.min)
                    sh = sg_p.tile([128, 4, 192], dt.bfloat16, tag="sh",
                                   name="sh")
                    nc.vector.tensor_sub(sh[:, 0:nq, :], o2t[:, 0:nq, :],
                                         cl[:, 0:nq, :])
                    # gate: y = s * u (complex), in [hf, c] layout, nq j's at once
                    _s = sh[:, :, :]
                    sr2 = _bass.AP(
                        tensor=_s.tensor, offset=_s.offset,
                        ap=[_s.ap[0], [192, nq], [0, 2], [1, BS]])
                    si2 = _bass.AP(
                        tensor=_s.tensor, offset=_s.offset + 96,
                        ap=[_s.ap[0], [192, nq], [0, 2], [1, BS]])
                    u2 = utT[:, :, j0:j0 + nq, :].rearrange(
                        "p r j c -> p j r c")
                    ta = sg_p.tile([128, 4, 2, BS], dt.bfloat16, tag="ta",
                                   name="ta")
                    tb = sg_p.tile([128, 4, 2, BS], dt.bfloat16, tag="tb",
                                   name="tb")
                    nc.vector.tensor_mul(ta[:, 0:nq, :, :], sr2, u2)
                    nc.vector.tensor_mul(tb[:, 0:nq, :, :], si2, u2)
                    nc.vector.tensor_sub(
                        ybig[:, 0, j0:j0 + nq, :],
                        ta[:, 0:nq, 0, :], tb[:, 0:nq, 1, :])
                    nc.gpsimd.tensor_add(
                        ybig[:, 1, j0:j0 + nq, :],
                        ta[:, 0:nq, 1, :], tb[:, 0:nq, 0, :])

                prev = None
                for ci in range(nchunk):
                    cur = mix1(ci)
                    if prev is not None:
                        mix2T(prev)
                    if weave is not None:
                        next(weave, None)
                        next(weave, None)
                    prev = cur
                mix2T(prev)
                if weave is not None:
                    for _ in weave:
                        pass
                if dbg and b == 0:
                    nc.sync.dma_start(out=d_ybig[:, :, :, :], in_=ybig[:, :, :, :])
                return ybig

            def idft_final(b, ybig, weave=None):
                def idftA(p):
                    pV = pN.tile([128, 2, 256], dt.float32, tag="pN", name="pV")
                    for cc in range(2):
                        c = 2 * p + cc
                        yr = ybig[:, 0, :, c]
                        yi = ybig[:, 1, :, c]
                        nc.tensor.matmul(pV[0:WF, cc, :], yr, mcs,
                                         start=(cc == 0), stop=False,
                                         skip_group_check=True)
                        nc.tensor.matmul(pV[0:WF, cc, :], yi, msc,
                                         start=False, stop=(cc == 1),
                                         skip_group_check=True)
                    return pV

                def vdrain(p, pV, vst):
                    # vst [65, 2(re/im), 4(c), 128(h)] fp8, scaled 1/1024 to
                    # undo ybig = 1024*y_ref; engine rotates ACT/DVE/Pool.
                    q = p % 2
                    dst = vst[0:WF, :, 2 * q:2 * q + 2, :]
                    src = pV[0:WF, :, :].rearrange("p c (r h) -> p r c h", r=2)
                    if p % 2 == 0:
                        nc.scalar.activation(dst, src, AF.Copy, bias=0.0,
                                             scale=1.0 / 1024.0)
                    else:
                        nc.vector.tensor_scalar(dst, src, 1.0 / 1024.0, None,
                                                ALU.mult)

                def final(qd, vst, xh4):
                    pO = pM.tile([128, 4, 128], dt.float32, tag="pM", name="pO")
                    mov = vst[:, :, :, :].rearrange("p r c h -> p r (c h)")
                    nc.tensor.matmul(pO[:, :, :].rearrange("p c f -> p (c f)"),
                                     fpair[:, :, :], mov,
                                     start=True, stop=True,
                                     perf_mode=DR, skip_group_check=True)
                    c0 = 4 * qd
                    ot = ot_p.tile([128, 4, 128], dt.bfloat16, tag="ot",
                                   name="ot")
                    nc.vector.tensor_add(
                        ot[:, :, :].rearrange("p c f -> p (c f)"),
                        pO[:, :, :].rearrange("p c f -> p (c f)"),
                        xh4.rearrange("p c f -> p (c f)"))
                    nc.sync.dma_start(
                        out=out[b, c0:c0 + 4, :, :].rearrange("c w h -> w c h"),
                        in_=ot[:, :, :])

                vsts = {}
                rxs = {}
                for p in range(BS // 2):
                    qd = p // 2
                    if p % 2 == 0:
                        vsts[qd] = vst_p.tile([WF, 2, 4, 128], dt.float8e4,
                                              tag="vst", name="vst")
                        # prefetch the residual x slab for this 4-channel group
                        rxs[qd] = rx_p.tile([128, 4, 128], dt.bfloat16,
                                            tag="rx", name="rx")
                        nc.sync.dma_start(out=rxs[qd][:],
                                          in_=xt[b, :, 4 * qd:4 * qd + 4, :])
                    pV = idftA(p)
                    vdrain(p, pV, vsts[qd])
                    if weave is not None:
                        next(weave, None)
                    if dbg and b == 0 and p == 1:
                        nc.sync.dma_start(out=d_vst[:, :, :, :], in_=vsts[0][:, :, :, :])
                    if p % 2 == 1 and p >= 3:
                        final(qd - 1, vsts[qd - 1], rxs.pop(qd - 1))
                final(BS // 4 - 1, vsts[BS // 4 - 1], rxs.pop(BS // 4 - 1))
                if weave is not None:
                    for _ in weave:
                        pass

            # ---- batch-level software pipeline, woven: the s1(b+1) groups
            # ---- are emitted between mix(b) chunks (PE work under the mix
            # ---- elementwise tail), and the s2(b+1) pairs between idft(b)
            # ---- iterations, so per-engine queues interleave finely.
            box = {}
            t8 = load_x(0)
            for _ in s1(0, t8, box):
                pass
            u1 = box["u1"]
            for _ in s2(0, u1, box):
                pass
            utr, utT = box["utr"], box["utT"]
            for b in range(B):
                s1g = None
                if b + 1 < B:
                    t8 = load_x(b + 1)
                    s1g = s1(b + 1, t8, box)
                ybig = mix(b, utr, utT, weave=s1g)
                s2g = None
                if b + 1 < B:
                    u1 = box["u1"]
                    s2g = s2(b + 1, u1, box)
                idft_final(b, ybig, weave=s2g)
                if b + 1 < B:
                    utr, utT = box["utr"], box["utT"]

    nc.finalize()
    return nc


_BUILT = None


def _get_built():
    global _BUILT
    if _BUILT is None:
        _BUILT = _build()
    return _BUILT


def _make_in_maps(x, w1, b1, w2, b2):
    fwpair, s2cs, s2sc, mcs, msc, fpair = _twiddles()
    in_maps = []
    for k in range(NBLK):
        xs = x[:, k * BS:(k + 1) * BS]
        w1r, w1i = w1[0, k], w1[1, k]
        w2r, w2i = w2[0, k], w2[1, k]

        def pad128(a):
            o = np.zeros((a.shape[0], 128), np.float32)
            o[:, 0:BS] = a
            return o.astype(BF16)

        # mix1 DoubleRow: psum = 64*(W1 xf + b1); weights 2*W1 keep the fp8
        # entries mostly normal-range, bias rows carry 64*b1
        inv = 1.0 / U_SCALE
        # mix2 DoubleRow pair: plane0 x o1-re, plane1 x o1-im; x32 so
        # sh = 32*sh_ref and the fp8 weights sit at ~0.6
        w2p0 = np.vstack([np.hstack([w2r, w2i]),
                          np.hstack([b2[0, k][None, :], b2[1, k][None, :]])])
        w2p1 = np.vstack([np.hstack([-w2i, w2r]), np.zeros((1, 192))])
        w2pair = (SH_SCALE * np.stack([w2p0, w2p1], axis=1)).astype(FP8)
        xsw = np.ascontiguousarray(xs.transpose(0, 3, 1, 2))  # [B, W, BS, H]
        m = {
            "xt": xsw.astype(BF16),
            "xt8": xsw.reshape(B, 2, 64, BS, 128).transpose(0, 2, 1, 3, 4
                                                            ).astype(FP8),
            "fwpair": fwpair, "s2cs": s2cs, "s2sc": s2sc,
            "mcs": mcs, "msc": msc, "fpair": fpair,
            "w1ra": pad128(np.vstack([inv * w1r, b1[0, k][None, :]])),
            "w1ia": pad128(np.vstack([inv * w1i, b1[1, k][None, :]])),
            "w1in": pad128(-inv * w1i), "w1r_": pad128(inv * w1r),
            "w2pair": w2pair,
        }
        in_maps.append(m)
    return in_maps


def kernel(x, w1, b1, w2, b2):
    from concourse.bass_utils import run_bass_kernel_spmd

    nc = _get_built()
    in_maps = _make_in_maps(x, w1, b1, w2, b2)

    trace = bool(int(os.environ.get("AFNO_TRACE", "0")))
    kw = {}
    if trace:
        import tempfile
        kw["tmpdir"] = tempfile.mkdtemp(prefix="afno_trace_")
        LAST_RESULT["trace_dir"] = kw["tmpdir"]
    res = run_bass_kernel_spmd(nc, in_maps, core_ids=list(range(NBLK)),
                               trace=trace, **kw)
    LAST_RESULT["exec_time_ns"] = res.exec_time_ns
    LAST_RESULT["results"] = res.results

    outp = np.empty((B, C, H, W), np.float32)
    for k in range(NBLK):
        outp[:, k * BS:(k + 1) * BS] = \
            res.results[k]["out"].astype(np.float32).transpose(0, 1, 3, 2)
    return outp
